# revision 5
# baseline (speedup 1.0000x reference)
"""Trainium2 Bass kernel for nn_CGPCoupler (sparse Clebsch-Gordan bilinear coupling).

Reference computation:
    out[:, ro] += x1[:, r1] * x2[:, r2] * cg        (nnz = 9856 sparse entries)

The index triples come in 16-wide aligned runs, so the op factors over 16-element
"subslots" (40 of them in the 640-dim rep space):

    out_O  +=  c_t * (x1_A (*) x2_B)     for 616 (A,B,O) terms, 308 distinct (A,B)

v4 dataflow ("square trick", all on-chip, data parallel over batch, 1024 rows/core):

    2*x1_a*x2_b = (x1_a + x2_b)^2 - x1_a^2 - x2_b^2

    1. G+  = SEL^T @ xs          TensorE one-hot sums -> PSUM. Each 128-row product
                                 chunk draws from a <=64-row source window, so the
                                 5 chunks run as K=64 row-tiles at strip 0/64
                                 (2-way concurrent in the PE array).
    2. P   = square(G+)          ScalarE ACT square + VectorE tensor_mul split the
                                 PSUM->SBUF fp16 pass (replaces the evacuate+multiply
                                 of the product formulation - and kills the 10.5 MB
                                 host-replicated x1g HBM stream of v3).
    3. sqE = xq (*) xq           GpSimd squares the compact 80-row inputs (fp16 SBUF)
                                 for the -x1^2 - x2^2 correction rows.
    4. out = W^T @ [P; sqE]      TensorE scatter, CG coeffs (c/2, and negated column
                                 sums for the corrections) folded into W, PSUM-
                                 accumulated over 7 chunks; evacuated as fp16.

Host-side numpy work (layout shuffles, building SEL/W/xs) is preprocessing of
inputs/constants; all arithmetic combining x1 and x2 happens on the NeuronCores.
"""

import os
import sys
import types

import numpy as np


def _ensure_ntff_hook():
    """concourse's trace path imports antenv.axon_hooks, which this image's
    antenv lacks. Provide it (and register the real profiling hook when the
    axon boot module is available) so tracing works instead of crashing."""
    try:
        import antenv
    except ImportError:
        return
    if getattr(antenv, "axon_hooks", None) is not None:
        return
    try:
        from antenv import axon_hooks  # noqa: F401
        return
    except ImportError:
        pass
    mod = types.ModuleType("antenv.axon_hooks")
    state = {"hook": None}
    mod.set_axon_ntff_profile_hook = lambda h: state.__setitem__("hook", h)
    mod.get_axon_ntff_profile_hook = lambda: state["hook"]
    sys.modules["antenv.axon_hooks"] = mod
    antenv.axon_hooks = mod
    try:
        from trn_agent_boot.trn_boot import _ntff_profile_via_ctypes
        so = "/opt/axon/libaxon_pjrt.so"
        if os.path.exists(so):
            mod.set_axon_ntff_profile_hook(_ntff_profile_via_ctypes(so))
    except Exception:
        pass


_ensure_ntff_hook()

N = 8192
DIM = 640
NCORES = 8
NLOC = N // NCORES          # rows per core
NSUB = DIM // 16            # 40 subslots
P_IN = NSUB * 2             # 80 half-rows: (subslot, ch-half)
CHH = 8                     # channels per half
FTOT = NLOC * CHH           # 8192 free elements per partition
FSUP = 1024                 # free-dim super-chunk
FCH = 512                   # free-dim chunk per matmul (one PSUM bank, fp32)
NCH = 5                     # product chunks (616 rows -> 5x128)
STRIP = 64                  # gather source window (row-tile granularity)
NQUART = 4                  # input DMA split along the free dim

LAST_RESULTS = None         # BassKernelResults of the most recent run

_plan_cache = {}
_program_cache = {}


def _build_plan(cg, r1, r2, ro):
    """Derive the chunked square-trick plan from the sparse index lists.

    Returns (src, SEL, W, WE0, WE1) where
      src  [NCH, STRIP, 2] int: (side, half-row) feeding strip slot i of chunk c
           (side 0 = x1, 1 = x2, -1 = unused)
      SEL  [NCH, STRIP, 128] f16: G+ gather weights (one-hot sums)
      W    [NCH, 128, P_IN] f16: scatter weights (c/2 per output of each product)
      WE0  [128, P_IN], WE1 [32, P_IN] f16: correction weights for
           [x1f^2; x2f[0:48]^2] and x2f[48:80]^2
    """
    key = (r1.tobytes(), r2.tobytes(), ro.tobytes(), cg.tobytes())
    hit = _plan_cache.get(key)
    if hit is not None:
        return hit

    A = r1 // 16
    B = r2 // 16
    O = ro // 16
    j = r1 % 16
    assert (r2 % 16 == j).all() and (ro % 16 == j).all(), \
        "index triples are not 16-aligned runs"
    assert A.max() < NSUB and B.max() < NSUB and O.max() < NSUB

    terms = {}   # (A,B,O) -> [coeff, covered-bitmask]
    for a, b, o, jj, c in zip(A.tolist(), B.tolist(), O.tolist(),
                              j.tolist(), cg.tolist()):
        k = (a, b, o)
        e = terms.get(k)
        if e is None:
            terms[k] = [c, 1 << jj]
        else:
            assert e[0] == c, "coefficient varies within a 16-run"
            assert not (e[1] >> jj) & 1, "duplicate (A,B,O,j) entry"
            e[1] |= 1 << jj
    for k, (c, mask) in terms.items():
        assert mask == 0xFFFF, f"term {k} covers only mask {mask:#x}"

    # products sorted for source-window locality; chunk = 64 products = 128 rows
    prods = sorted({(a, b) for (a, b, o) in terms},
                   key=lambda ab: (max(ab), min(ab)))
    D = len(prods)
    assert D <= NCH * 64

    outs = {}    # (A,B) -> [(O, c), ...]
    for (a, b, o), (c, _) in terms.items():
        outs.setdefault((a, b), []).append((o, c))

    src = np.full((NCH, STRIP, 2), -1, np.int64)
    SEL = np.zeros((NCH, STRIP, 128), np.float16)
    W = np.zeros((NCH, 128, P_IN), np.float16)
    for cix in range(NCH):
        chunk = prods[cix * 64:(cix + 1) * 64]
        slot = {}     # (side, half-row) -> strip index
        for d, (a, b) in enumerate(chunk):
            for hh in (0, 1):
                row = 2 * d + hh
                for side, sub in ((0, a), (1, b)):
                    k = (side, 2 * sub + hh)
                    i = slot.get(k)
                    if i is None:
                        i = len(slot)
                        assert i < STRIP, f"chunk {cix} window > {STRIP}"
                        slot[k] = i
                        src[cix, i] = k
                    SEL[cix, i, row] = 1.0
                for o, c in outs[(a, b)]:
                    W[cix, row, 2 * o + hh] = c / 2.0

    # correction weights: out -= sum_a S1[a,:] x1_a^2 + sum_b S2[b,:] x2_b^2
    S1 = np.zeros((P_IN, P_IN), np.float64)
    S2 = np.zeros((P_IN, P_IN), np.float64)
    for (a, b, o), (c, _) in terms.items():
        for hh in (0, 1):
            S1[2 * a + hh, 2 * o + hh] += c / 2.0
            S2[2 * b + hh, 2 * o + hh] += c / 2.0
    SE = -np.concatenate([S1, S2])          # [160, 80]
    WE0 = SE[:128].astype(np.float16)
    WE1 = SE[128:].astype(np.float16)       # [32, 80]

    out = (src, SEL, W, WE0, WE1)
    _plan_cache[key] = out
    return out


def _pack_x(x):
    """[NLOC, 640] -> [80, NLOC*8] fp16: row p = subslot*2 + half, col = n*8 + ch."""
    return np.ascontiguousarray(
        x.reshape(NLOC, NSUB, 2, CHH).transpose(1, 2, 0, 3).reshape(P_IN, FTOT),
        dtype=np.float16)


def _unpack_out(o):
    """[80, NLOC*8] -> [NLOC, 640]."""
    return o.reshape(NSUB, 2, NLOC, CHH).transpose(2, 0, 1, 3).reshape(NLOC, DIM)


def _build_xs(x1f, x2f, src):
    """Gather source strips [NCH, STRIP, FTOT] fp16 per the plan's src map."""
    xs = np.zeros((NCH, STRIP, FTOT), np.float16)
    both = (x1f, x2f)
    for c in range(NCH):
        for i in range(STRIP):
            side, row = src[c, i]
            if side >= 0:
                xs[c, i] = both[side][row]
    return xs


def _build_program():
    """v4: square-trick dataflow (see module docstring)."""
    import concourse.mybir as mybir
    import concourse.tile as tile
    from concourse import bacc
    from concourse.bass import ds, ts

    f32 = mybir.dt.float32
    f16 = mybir.dt.float16
    nc = bacc.Bacc("TRN2", target_bir_lowering=False)

    NSUP = FTOT // FSUP     # 8
    NJ = FSUP // FCH        # 2 matmul FD chunks per super-chunk
    FQ = FTOT // NQUART     # free-dim quarter per input dma
    SPQ = NSUP // NQUART    # supers per quarter

    # gather sources, [128, FTOT] each:
    #   xab: chunk0 strips at partitions 0:64, chunk1 at 64:128
    #   xcd: chunks 2,3; xet: chunk4 at 0:64, x2f[48:80] at 64:96
    #   xq:  x1f (80 rows) + x2f[0:48] at 80:128
    srcs = [nc.dram_tensor(n_, [128, FTOT], f16, kind="ExternalInput")
            for n_ in ("xab", "xcd", "xet", "xq")]
    seld = nc.dram_tensor("sel", [128, 3 * 128], f16, kind="ExternalInput")
    wd = nc.dram_tensor("wmat", [128, (NCH + 2) * P_IN], f16, kind="ExternalInput")
    outd = nc.dram_tensor("outf", [P_IN, FTOT], f16, kind="ExternalOutput")

    with tile.TileContext(nc) as tc:
        with tc.tile_pool(name="const", bufs=1) as constp, \
             tc.tile_pool(name="sq", bufs=3) as sqp, \
             tc.tile_pool(name="psb", bufs=2 * NCH) as psb, \
             tc.tile_pool(name="og", bufs=4) as og, \
             tc.tile_pool(name="psg", bufs=2, space="PSUM") as psg, \
             tc.tile_pool(name="pso", bufs=4, space="PSUM") as pso:

            # constants: SEL strips packed as [128, 3*128]: col-block k holds
            # chunks (2k, 2k+1) at partition strips (0, 64); k=2 holds chunk 4.
            sel = constp.tile([128, 3 * 128], f16, tag="sel")
            nc.sync.dma_start(out=sel, in_=seld[:])
            w = constp.tile([128, (NCH + 2) * P_IN], f16, tag="w")
            nc.sync.dma_start(out=w, in_=wd[:])

            # source quarters: separate tiles so early supers don't wait on the
            # whole 8 MB input load
            qt = {}     # (tensor_idx, quarter) -> tile
            for q in range(NQUART):
                for i, dram in enumerate(srcs):
                    t = constp.tile([128, FQ], f16, tag=f"src{i}q{q}")
                    nc.sync.dma_start(out=t, in_=dram[:, ds(q * FQ, FQ)])
                    qt[(i, q)] = t

            def scatter(sup, pts, sqe0, sqe1, outps):
                for jj in range(NJ):
                    for c in range(NCH):
                        nc.tensor.matmul(outps[jj], w[:, ts(c, P_IN)],
                                         pts[c][:, ts(jj, FCH)],
                                         start=(c == 0), stop=False,
                                         skip_group_check=True)
                    nc.tensor.matmul(outps[jj], w[:, ts(NCH, P_IN)],
                                     sqe0[:, ts(jj, FCH)],
                                     start=False, stop=False,
                                     skip_group_check=True)
                    nc.tensor.matmul(outps[jj], w[ds(64, 32), ts(NCH + 1, P_IN)],
                                     sqe1[ds(64, 32), ts(jj, FCH)],
                                     start=False, stop=True,
                                     skip_group_check=True)
                outt = og.tile([P_IN, FSUP], f16, tag="outt")
                nc.vector.tensor_copy(out=outt[:, ts(0, FCH)], in_=outps[0])
                nc.scalar.copy(out=outt[:, ts(1, FCH)], in_=outps[1])
                nc.sync.dma_start(out=outd[:, ds(sup * FSUP, FSUP)], in_=outt)

            prev = None
            for sup in range(NSUP):
                q, so = sup // SPQ, (sup % SPQ) * FSUP
                ssl = ds(so, FSUP)
                xab, xcd, xet, xq = (qt[(i, q)] for i in range(4))

                # correction squares on GpSimd (fp16 SBUF)
                sqe0 = sqp.tile([128, FSUP], f16, tag="sqe0")
                nc.gpsimd.tensor_mul(sqe0, xq[:, ssl], xq[:, ssl])
                sqe1 = sqp.tile([128, FSUP], f16, tag="sqe1")
                nc.gpsimd.tensor_mul(sqe1[ds(64, 32)], xet[ds(64, 32), ssl],
                                     xet[ds(64, 32), ssl])

                # gathers (K=64 row-tiles, strips alternate 0/64) + squares
                pts = []
                for c in range(NCH):
                    xt = (xab, xcd, xet)[c // 2]
                    base = STRIP * (c % 2)
                    gp = psg.tile([128, FSUP], f32, tag="gp")
                    for jj in range(NJ):
                        nc.tensor.matmul(
                            gp[:, ts(jj, FCH)],
                            sel[ds(base, STRIP), ts(c // 2, 128)],
                            xt[ds(base, STRIP), so + jj * FCH:
                               so + (jj + 1) * FCH],
                            start=True, stop=True)
                    pt = psb.tile([128, FSUP], f16, tag="pt")
                    # split the PSUM->SBUF squaring pass (V cannot read two PSUM
                    # operands): S squares 3 chunks straight from PSUM; for the
                    # other 2, V evacuates fp16 and GpSimd squares from SBUF
                    if c % 2 == 0:
                        nc.scalar.square(out=pt, in_=gp)
                    else:
                        gpt = sqp.tile([128, FSUP], f16, tag="gpt")
                        nc.vector.tensor_copy(out=gpt, in_=gp)
                        nc.gpsimd.tensor_mul(pt, gpt, gpt)
                    pts.append(pt)

                # scatter the PREVIOUS super while this super's squares drain:
                # PE order g(0) g(1) s(0) g(2) s(1) ... keeps the array busy
                if prev is not None:
                    scatter(*prev)
                outps = [pso.tile([P_IN, FCH], f32, tag="outp",
                                  name=f"outp{sup}_{jj}")
                         for jj in range(NJ)]
                prev = (sup, pts, sqe0, sqe1, outps)

            scatter(*prev)

    nc.compile()
    return nc


def kernel(x1, x2, cg_tilde, repids_in1, repids_in2, repids_out, out_dim=DIM,
           **_ignored):
    global LAST_RESULTS
    import concourse.bass_utils as _bu
    from concourse.bass_utils import run_bass_kernel_spmd
    # the trace path uploads artifacts to S3, which this container can't reach
    if not getattr(_bu.upload_artifacts, "_local", False):
        _bu.upload_artifacts = lambda tmpdir: "local://" + tmpdir
        _bu.upload_artifacts._local = True

    x1 = np.ascontiguousarray(np.asarray(x1), dtype=np.float32)
    x2 = np.ascontiguousarray(np.asarray(x2), dtype=np.float32)
    cg = np.asarray(cg_tilde, dtype=np.float32)
    r1 = np.asarray(repids_in1, dtype=np.int64)
    r2 = np.asarray(repids_in2, dtype=np.int64)
    ro = np.asarray(repids_out, dtype=np.int64)
    out_dim = int(out_dim)
    assert x1.shape == (N, DIM) and x2.shape == (N, DIM) and out_dim == DIM

    src, SEL, W, WE0, WE1 = _build_plan(cg, r1, r2, ro)

    nc = _program_cache.get("v4")
    if nc is None:
        nc = _build_program()
        _program_cache["v4"] = nc

    # pack constants
    selp = np.zeros((128, 3 * 128), np.float16)
    for c in range(NCH):
        base = STRIP * (c % 2)
        selp[base:base + STRIP, (c // 2) * 128:(c // 2 + 1) * 128] = SEL[c]
    wp = np.zeros((128, (NCH + 2) * P_IN), np.float16)
    for c in range(NCH):
        wp[:, c * P_IN:(c + 1) * P_IN] = W[c]
    wp[:, NCH * P_IN:(NCH + 1) * P_IN] = WE0
    wp[64:96, (NCH + 1) * P_IN:(NCH + 2) * P_IN] = WE1

    in_maps = []
    for cr in range(NCORES):
        sl = slice(cr * NLOC, (cr + 1) * NLOC)
        x1f = _pack_x(x1[sl])
        x2f = _pack_x(x2[sl])
        xs = _build_xs(x1f, x2f, src)          # [5, 64, FTOT]
        xab = np.ascontiguousarray(xs[0:2].reshape(128, FTOT))
        xcd = np.ascontiguousarray(xs[2:4].reshape(128, FTOT))
        xet = np.zeros((128, FTOT), np.float16)
        xet[0:64] = xs[4]
        xet[64:96] = x2f[48:80]
        xq = np.zeros((128, FTOT), np.float16)
        xq[0:80] = x1f
        xq[80:128] = x2f[0:48]
        in_maps.append({
            "xab": xab, "xcd": xcd, "xet": xet, "xq": xq,
            "sel": selp, "wmat": wp,
        })

    res = run_bass_kernel_spmd(nc, in_maps, core_ids=list(range(NCORES)))
    LAST_RESULTS = res

    out = np.empty((N, DIM), np.float32)
    for cr in range(NCORES):
        out[cr * NLOC:(cr + 1) * NLOC] = _unpack_out(
            np.asarray(res.results[cr]["outf"], dtype=np.float32))
    return out


def _numpy_model(x1, x2, cg, r1, r2, ro):
    """Host-side model of the device dataflow (including fp16 quantization),
    for validating index logic and predicting the on-device error."""
    src, SEL, W, WE0, WE1 = _build_plan(cg, r1, r2, ro)
    out = np.empty_like(x1)
    for cr in range(NCORES):
        sl = slice(cr * NLOC, (cr + 1) * NLOC)
        x1f = _pack_x(x1[sl])
        x2f = _pack_x(x2[sl])
        xs = _build_xs(x1f, x2f, src)
        outf = np.zeros((P_IN, FTOT), np.float32)
        for c in range(NCH):
            gp = SEL[c].astype(np.float32).T @ xs[c].astype(np.float32)
            pt = (gp * gp).astype(np.float16).astype(np.float32)
            outf += W[c].astype(np.float32).T @ pt
        xqf = np.concatenate([x1f, x2f]).astype(np.float32)   # [160, FTOT]
        sq = (xqf * xqf).astype(np.float16).astype(np.float32)
        outf += WE0.astype(np.float32).T @ sq[:128]
        outf += WE1.astype(np.float32).T @ sq[128:]
        out[sl] = _unpack_out(outf.astype(np.float16).astype(np.float32))
    return out


# revision 9
# speedup vs baseline: 1.3274x; 1.3274x over previous
"""Trainium2 Bass kernel for nn_CGPCoupler (sparse Clebsch-Gordan bilinear coupling).

Reference computation:
    out[:, ro] += x1[:, r1] * x2[:, r2] * cg        (nnz = 9856 sparse entries)

The index triples come in 16-wide aligned runs, so the op factors over 16-element
"subslots" (40 of them in the 640-dim rep space):

    out_O  +=  c_t * (x1_A (*) x2_B)     for 616 (A,B,O) terms, 308 distinct (A,B)

v4 dataflow ("square trick", all on-chip, data parallel over batch, 1024 rows/core):

    2*x1_a*x2_b = (x1_a + x2_b)^2 - x1_a^2 - x2_b^2

    1. G+  = SEL^T @ xs          TensorE one-hot sums -> PSUM. Each 128-row product
                                 chunk draws from a <=64-row source window, so the
                                 5 chunks run as K=64 row-tiles at strip 0/64
                                 (2-way concurrent in the PE array).
    2. P   = square(G+)          ScalarE ACT square + VectorE tensor_mul split the
                                 PSUM->SBUF fp16 pass (replaces the evacuate+multiply
                                 of the product formulation - and kills the 10.5 MB
                                 host-replicated x1g HBM stream of v3).
    3. sqE = xq (*) xq           GpSimd squares the compact 80-row inputs (fp16 SBUF)
                                 for the -x1^2 - x2^2 correction rows.
    4. out = W^T @ [P; sqE]      TensorE scatter, CG coeffs (c/2, and negated column
                                 sums for the corrections) folded into W, PSUM-
                                 accumulated over 7 chunks; evacuated as fp16.

Host-side numpy work (layout shuffles, building SEL/W/xs) is preprocessing of
inputs/constants; all arithmetic combining x1 and x2 happens on the NeuronCores.
"""

import os
import sys
import types

import numpy as np


def _ensure_ntff_hook():
    """concourse's trace path imports antenv.axon_hooks, which this image's
    antenv lacks. Provide it (and register the real profiling hook when the
    axon boot module is available) so tracing works instead of crashing."""
    try:
        import antenv
    except ImportError:
        return
    if getattr(antenv, "axon_hooks", None) is not None:
        return
    try:
        from antenv import axon_hooks  # noqa: F401
        return
    except ImportError:
        pass
    mod = types.ModuleType("antenv.axon_hooks")
    state = {"hook": None}
    mod.set_axon_ntff_profile_hook = lambda h: state.__setitem__("hook", h)
    mod.get_axon_ntff_profile_hook = lambda: state["hook"]
    sys.modules["antenv.axon_hooks"] = mod
    antenv.axon_hooks = mod
    try:
        from trn_agent_boot.trn_boot import _ntff_profile_via_ctypes
        so = "/opt/axon/libaxon_pjrt.so"
        if os.path.exists(so):
            mod.set_axon_ntff_profile_hook(_ntff_profile_via_ctypes(so))
    except Exception:
        pass


_ensure_ntff_hook()

N = 8192
DIM = 640
NCORES = 8
NLOC = N // NCORES          # rows per core
NSUB = DIM // 16            # 40 subslots
P_IN = NSUB * 2             # 80 half-rows: (subslot, ch-half)
CHH = 8                     # channels per half
FTOT = NLOC * CHH           # 8192 free elements per partition
FSUP = 1024                 # free-dim super-chunk
FCH = 512                   # free-dim chunk per matmul (one PSUM bank, fp32)
NCH = 5                     # product chunks (616 rows -> 5x128)
STRIP = 64                  # gather source window (row-tile granularity)
NQUART = 4                  # input DMA split along the free dim

LAST_RESULTS = None         # BassKernelResults of the most recent run

_plan_cache = {}
_program_cache = {}


def _build_plan(cg, r1, r2, ro):
    """Derive the chunked square-trick plan from the sparse index lists.

    Returns (src, SEL, W, WE0, WE1) where
      src  [NCH, STRIP, 2] int: (side, half-row) feeding strip slot i of chunk c
           (side 0 = x1, 1 = x2, -1 = unused)
      SEL  [NCH, STRIP, 128] f16: G+ gather weights (one-hot sums)
      W    [NCH, 128, P_IN] f16: scatter weights (c/2 per output of each product)
      WE0  [128, P_IN], WE1 [32, P_IN] f16: correction weights for
           [x1f^2; x2f[0:48]^2] and x2f[48:80]^2
    """
    key = (r1.tobytes(), r2.tobytes(), ro.tobytes(), cg.tobytes())
    hit = _plan_cache.get(key)
    if hit is not None:
        return hit

    A = r1 // 16
    B = r2 // 16
    O = ro // 16
    j = r1 % 16
    assert (r2 % 16 == j).all() and (ro % 16 == j).all(), \
        "index triples are not 16-aligned runs"
    assert A.max() < NSUB and B.max() < NSUB and O.max() < NSUB

    terms = {}   # (A,B,O) -> [coeff, covered-bitmask]
    for a, b, o, jj, c in zip(A.tolist(), B.tolist(), O.tolist(),
                              j.tolist(), cg.tolist()):
        k = (a, b, o)
        e = terms.get(k)
        if e is None:
            terms[k] = [c, 1 << jj]
        else:
            assert e[0] == c, "coefficient varies within a 16-run"
            assert not (e[1] >> jj) & 1, "duplicate (A,B,O,j) entry"
            e[1] |= 1 << jj
    for k, (c, mask) in terms.items():
        assert mask == 0xFFFF, f"term {k} covers only mask {mask:#x}"

    # products sorted for source-window locality; chunk = 64 products = 128 rows
    prods = sorted({(a, b) for (a, b, o) in terms},
                   key=lambda ab: (max(ab), min(ab)))
    D = len(prods)
    assert D <= NCH * 64

    outs = {}    # (A,B) -> [(O, c), ...]
    for (a, b, o), (c, _) in terms.items():
        outs.setdefault((a, b), []).append((o, c))

    src = np.full((NCH, STRIP, 2), -1, np.int64)
    SEL = np.zeros((NCH, STRIP, 128), np.float16)
    W = np.zeros((NCH, 128, P_IN), np.float16)
    for cix in range(NCH):
        chunk = prods[cix * 64:(cix + 1) * 64]
        slot = {}     # (side, half-row) -> strip index
        for d, (a, b) in enumerate(chunk):
            for hh in (0, 1):
                row = 2 * d + hh
                for side, sub in ((0, a), (1, b)):
                    k = (side, 2 * sub + hh)
                    i = slot.get(k)
                    if i is None:
                        i = len(slot)
                        assert i < STRIP, f"chunk {cix} window > {STRIP}"
                        slot[k] = i
                        src[cix, i] = k
                    SEL[cix, i, row] = 1.0
                for o, c in outs[(a, b)]:
                    W[cix, row, 2 * o + hh] = c / 2.0

    # correction weights: out -= sum_a S1[a,:] x1_a^2 + sum_b S2[b,:] x2_b^2
    S1 = np.zeros((P_IN, P_IN), np.float64)
    S2 = np.zeros((P_IN, P_IN), np.float64)
    for (a, b, o), (c, _) in terms.items():
        for hh in (0, 1):
            S1[2 * a + hh, 2 * o + hh] += c / 2.0
            S2[2 * b + hh, 2 * o + hh] += c / 2.0
    SE = -np.concatenate([S1, S2])          # [160, 80]
    WE0 = SE[:128].astype(np.float16)
    WE1 = SE[128:].astype(np.float16)       # [32, 80]

    out = (src, SEL, W, WE0, WE1)
    _plan_cache[key] = out
    return out


def _pack_x(x):
    """[NLOC, 640] -> [80, NLOC*8] fp16: row p = subslot*2 + half, col = n*8 + ch."""
    return np.ascontiguousarray(
        x.reshape(NLOC, NSUB, 2, CHH).transpose(1, 2, 0, 3).reshape(P_IN, FTOT),
        dtype=np.float16)


def _unpack_out(o):
    """[80, NLOC*8] -> [NLOC, 640]."""
    return o.reshape(NSUB, 2, NLOC, CHH).transpose(2, 0, 1, 3).reshape(NLOC, DIM)


def _build_xs(x1f, x2f, src):
    """Gather source strips [NCH, STRIP, FTOT] fp16 per the plan's src map."""
    xs = np.zeros((NCH, STRIP, FTOT), np.float16)
    both = (x1f, x2f)
    for c in range(NCH):
        for i in range(STRIP):
            side, row = src[c, i]
            if side >= 0:
                xs[c, i] = both[side][row]
    return xs


def _build_program():
    """v4: square-trick dataflow (see module docstring)."""
    import concourse.mybir as mybir
    import concourse.tile as tile
    from concourse import bacc
    from concourse.bass import ds, ts

    f32 = mybir.dt.float32
    f16 = mybir.dt.float16
    nc = bacc.Bacc("TRN2", target_bir_lowering=False)

    NSUP = FTOT // FSUP     # 8
    NJ = FSUP // FCH        # 2 matmul FD chunks per super-chunk
    FQ = FTOT // NQUART     # free-dim quarter per input dma
    SPQ = NSUP // NQUART    # supers per quarter

    # gather sources, [128, FTOT] each:
    #   xab: chunk0 strips at partitions 0:64, chunk1 at 64:128
    #   xcd: chunks 2,3; xet: chunk4 at 0:64, x2f[48:80] at 64:96
    #   xq:  x1f (80 rows) + x2f[0:48] at 80:128
    srcs = [nc.dram_tensor(n_, [128, FTOT], f16, kind="ExternalInput")
            for n_ in ("xab", "xcd", "xet", "xq")]
    seld = nc.dram_tensor("sel", [128, 3 * 128], f16, kind="ExternalInput")
    wd = nc.dram_tensor("wmat", [128, (NCH + 2) * P_IN], f16, kind="ExternalInput")
    outd = nc.dram_tensor("outf", [P_IN, FTOT], f16, kind="ExternalOutput")

    with tile.TileContext(nc) as tc:
        with tc.tile_pool(name="const", bufs=1) as constp, \
             tc.tile_pool(name="sq", bufs=3) as sqp, \
             tc.tile_pool(name="psb", bufs=2 * NCH) as psb, \
             tc.tile_pool(name="og", bufs=4) as og, \
             tc.tile_pool(name="psg", bufs=3, space="PSUM") as psg, \
             tc.tile_pool(name="pso", bufs=2, space="PSUM") as pso:

            # constants: SEL strips packed as [128, 3*128]: col-block k holds
            # chunks (2k, 2k+1) at partition strips (0, 64); k=2 holds chunk 4.
            sel = constp.tile([128, 3 * 128], f16, tag="sel")
            nc.sync.dma_start(out=sel, in_=seld[:])
            w = constp.tile([128, (NCH + 2) * P_IN], f16, tag="w")
            nc.sync.dma_start(out=w, in_=wd[:])

            # source quarters: separate tiles so early supers don't wait on the
            # whole 8 MB input load
            qt = {}     # (tensor_idx, quarter) -> tile
            for q in range(NQUART):
                for i, dram in enumerate(srcs):
                    t = constp.tile([128, FQ], f16, tag=f"src{i}q{q}")
                    nc.sync.dma_start(out=t, in_=dram[:, ds(q * FQ, FQ)])
                    qt[(i, q)] = t

            def scatter(sup, pts, sqe0, sqe1, outps):
                for jj in range(NJ):
                    for c in range(NCH):
                        nc.tensor.matmul(outps[jj], w[:, ts(c, P_IN)],
                                         pts[c][:, ts(jj, FCH)],
                                         start=(c == 0), stop=False,
                                         skip_group_check=True)
                    nc.tensor.matmul(outps[jj], w[:, ts(NCH, P_IN)],
                                     sqe0[:, ts(jj, FCH)],
                                     start=False, stop=False,
                                     skip_group_check=True)
                    nc.tensor.matmul(outps[jj], w[ds(64, 32), ts(NCH + 1, P_IN)],
                                     sqe1[ds(64, 32), ts(jj, FCH)],
                                     start=False, stop=True,
                                     skip_group_check=True)
                outt = og.tile([P_IN, FSUP], f16, tag="outt")
                nc.vector.tensor_copy(out=outt[:, ts(0, FCH)], in_=outps[0])
                nc.scalar.copy(out=outt[:, ts(1, FCH)], in_=outps[1])
                nc.sync.dma_start(out=outd[:, ds(sup * FSUP, FSUP)], in_=outt)

            prev = None
            for sup in range(NSUP):
                q, so = sup // SPQ, (sup % SPQ) * FSUP
                ssl = ds(so, FSUP)
                xab, xcd, xet, xq = (qt[(i, q)] for i in range(4))

                # correction squares on GpSimd (fp16 SBUF)
                sqe0 = sqp.tile([128, FSUP], f16, tag="sqe0")
                nc.gpsimd.tensor_mul(sqe0, xq[:, ssl], xq[:, ssl])
                sqe1 = sqp.tile([128, FSUP], f16, tag="sqe1")
                nc.gpsimd.tensor_mul(sqe1[ds(64, 32)], xet[ds(64, 32), ssl],
                                     xet[ds(64, 32), ssl])

                # gathers (K=64 row-tiles, strips alternate 0/64) + squares
                pts = []
                for c in range(NCH):
                    xt = (xab, xcd, xet)[c // 2]
                    base = STRIP * (c % 2)
                    gp = psg.tile([128, FSUP], f32, tag="gp")
                    for jj in range(NJ):
                        nc.tensor.matmul(
                            gp[:, ts(jj, FCH)],
                            sel[ds(base, STRIP), ts(c // 2, 128)],
                            xt[ds(base, STRIP), so + jj * FCH:
                               so + (jj + 1) * FCH],
                            start=True, stop=True)
                    pt = psb.tile([128, FSUP], f16, tag="pt")
                    # split the PSUM->SBUF squaring pass (V cannot read two PSUM
                    # operands, and GpSimd TT is slow ~1.9us): S squares chunks
                    # 0-2 straight from PSUM; V evacuates fp16 + self-squares
                    # (2x mode) chunks 3-4, which the scatter consumes last
                    if c < 3:
                        nc.scalar.square(out=pt, in_=gp)
                    else:
                        gpt = sqp.tile([128, FSUP], f16, tag="gpt")
                        nc.vector.tensor_copy(out=gpt, in_=gp)
                        nc.vector.tensor_mul(pt, gpt, gpt)
                    pts.append(pt)

                # scatter the PREVIOUS super while this super's squares drain:
                # PE order g(0) g(1) s(0) g(2) s(1) ... keeps the array busy
                if prev is not None:
                    scatter(*prev)
                outps = [pso.tile([P_IN, FCH], f32, tag="outp",
                                  name=f"outp{sup}_{jj}")
                         for jj in range(NJ)]
                prev = (sup, pts, sqe0, sqe1, outps)

            scatter(*prev)

    nc.compile()
    return nc


def kernel(x1, x2, cg_tilde, repids_in1, repids_in2, repids_out, out_dim=DIM,
           **_ignored):
    global LAST_RESULTS
    import concourse.bass_utils as _bu
    from concourse.bass_utils import run_bass_kernel_spmd
    # the trace path uploads artifacts to S3, which this container can't reach
    if not getattr(_bu.upload_artifacts, "_local", False):
        _bu.upload_artifacts = lambda tmpdir: "local://" + tmpdir
        _bu.upload_artifacts._local = True

    x1 = np.ascontiguousarray(np.asarray(x1), dtype=np.float32)
    x2 = np.ascontiguousarray(np.asarray(x2), dtype=np.float32)
    cg = np.asarray(cg_tilde, dtype=np.float32)
    r1 = np.asarray(repids_in1, dtype=np.int64)
    r2 = np.asarray(repids_in2, dtype=np.int64)
    ro = np.asarray(repids_out, dtype=np.int64)
    out_dim = int(out_dim)
    assert x1.shape == (N, DIM) and x2.shape == (N, DIM) and out_dim == DIM

    src, SEL, W, WE0, WE1 = _build_plan(cg, r1, r2, ro)

    nc = _program_cache.get("v4")
    if nc is None:
        nc = _build_program()
        _program_cache["v4"] = nc

    # pack constants
    selp = np.zeros((128, 3 * 128), np.float16)
    for c in range(NCH):
        base = STRIP * (c % 2)
        selp[base:base + STRIP, (c // 2) * 128:(c // 2 + 1) * 128] = SEL[c]
    wp = np.zeros((128, (NCH + 2) * P_IN), np.float16)
    for c in range(NCH):
        wp[:, c * P_IN:(c + 1) * P_IN] = W[c]
    wp[:, NCH * P_IN:(NCH + 1) * P_IN] = WE0
    wp[64:96, (NCH + 1) * P_IN:(NCH + 2) * P_IN] = WE1

    in_maps = []
    for cr in range(NCORES):
        sl = slice(cr * NLOC, (cr + 1) * NLOC)
        x1f = _pack_x(x1[sl])
        x2f = _pack_x(x2[sl])
        xs = _build_xs(x1f, x2f, src)          # [5, 64, FTOT]
        xab = np.ascontiguousarray(xs[0:2].reshape(128, FTOT))
        xcd = np.ascontiguousarray(xs[2:4].reshape(128, FTOT))
        xet = np.zeros((128, FTOT), np.float16)
        xet[0:64] = xs[4]
        xet[64:96] = x2f[48:80]
        xq = np.zeros((128, FTOT), np.float16)
        xq[0:80] = x1f
        xq[80:128] = x2f[0:48]
        in_maps.append({
            "xab": xab, "xcd": xcd, "xet": xet, "xq": xq,
            "sel": selp, "wmat": wp,
        })

    res = run_bass_kernel_spmd(nc, in_maps, core_ids=list(range(NCORES)))
    LAST_RESULTS = res

    out = np.empty((N, DIM), np.float32)
    for cr in range(NCORES):
        out[cr * NLOC:(cr + 1) * NLOC] = _unpack_out(
            np.asarray(res.results[cr]["outf"], dtype=np.float32))
    return out


def _numpy_model(x1, x2, cg, r1, r2, ro):
    """Host-side model of the device dataflow (including fp16 quantization),
    for validating index logic and predicting the on-device error."""
    src, SEL, W, WE0, WE1 = _build_plan(cg, r1, r2, ro)
    out = np.empty_like(x1)
    for cr in range(NCORES):
        sl = slice(cr * NLOC, (cr + 1) * NLOC)
        x1f = _pack_x(x1[sl])
        x2f = _pack_x(x2[sl])
        xs = _build_xs(x1f, x2f, src)
        outf = np.zeros((P_IN, FTOT), np.float32)
        for c in range(NCH):
            gp = SEL[c].astype(np.float32).T @ xs[c].astype(np.float32)
            pt = (gp * gp).astype(np.float16).astype(np.float32)
            outf += W[c].astype(np.float32).T @ pt
        xqf = np.concatenate([x1f, x2f]).astype(np.float32)   # [160, FTOT]
        sq = (xqf * xqf).astype(np.float16).astype(np.float32)
        outf += WE0.astype(np.float32).T @ sq[:128]
        outf += WE1.astype(np.float32).T @ sq[128:]
        out[sl] = _unpack_out(outf.astype(np.float16).astype(np.float32))
    return out
